# revision 1
# baseline (speedup 1.0000x reference)
"""Capsule routing kernel for Trainium2 (8 NeuronCores, batch-sharded).

Math (per example b):
  u_hat[i, o] = sum_f x[b, i, f] * W[o, f]           (o = (n, d), 25 = 5x5)
  blog = 0
  for t in 0..3:
      c = softmax_n(blog)                            (c0 uniform = 1/5)
      outputs[n, d] = squash_d(sum_i c[n, i] * u_hat[i, (n, d)])
      if t < 3: blog[n, i] += sum_d outputs[n, d] * u_hat[i, (n, d)]
  return outputs                                     [B, 5, 5]

Plan per core (1024 examples):
  Phase A (TensorE): x tiles [128 bi, 80 f] -> transpose-matmul (rhs=I128)
    -> PSUM xT [80, 128] -> ACT evict -> SBUF -> matmul lhsT=xT, rhs=Wt[80,25]
    -> PSUM u_hat [128 bi, 25] -> ACT evict (16 tiles/bank) -> DRAM scratch.
  Regroup: DRAM scratch laid out [p, k, o]; read back per 128-example block
    as [b partitions, o-major free 25*256] (i-order permuted, harmless).
  Phase B (DVE + ACT): routing with one example per partition lane.
"""

import sys
import numpy as np

sys.path.insert(0, "/opt/trn_rl_repo")

NCORES = 8
B_FULL = 8192
I = 256
F = 80
O = 25  # NUM_CAPSULE * DIM_CAPSULE
N = 5
D = 5
ROUTINGS = 4
EPS = 1e-7

_CACHE = {}


def _build(b_core, stop_after="full"):
    from contextlib import ExitStack
    from concourse import bacc, mybir
    from concourse.tile import TileContext

    f32 = mybir.dt.float32
    Alu = mybir.AluOpType
    Act = mybir.ActivationFunctionType

    nblk = b_core // 128          # 128-example blocks
    tiles_blk = 256               # [128, 80] bi-tiles per block (128*256/128)
    k_total = b_core * I // 128   # total bi-tiles per core
    CH = 64                       # x tiles per input DMA chunk (2.6 MB)
    EV_A = 16                     # A-matmuls per psum bank (16*25=400 <= 512)
    STG = 64                      # tiles per scratch-write DMA (4 psum banks)

    nc = bacc.Bacc("TRN2", target_bir_lowering=False, debug=False)
    x_d = nc.dram_tensor("x", [b_core, I, F], f32, kind="ExternalInput")
    wt_d = nc.dram_tensor("wt", [F, O], f32, kind="ExternalInput")
    id_d = nc.dram_tensor("ident", [128, 128], f32, kind="ExternalInput")
    out_d = nc.dram_tensor("out", [b_core, O], f32, kind="ExternalOutput")
    # u_hat scratch: [p, k, o] = u_hat row bi = k*128+p, i.e. example
    # b = k//2, i = 128*(k%2)+p. Readback per block coalesces to 3-dim APs.
    us_d = nc.dram_tensor("uscr", [128, k_total, O], f32)

    with TileContext(nc) as tc, ExitStack() as ex:
        cst = ex.enter_context(tc.tile_pool(name="cst", bufs=1))
        xin = ex.enter_context(tc.tile_pool(name="xin", bufs=3))
        xtp = ex.enter_context(tc.tile_pool(name="xtp", bufs=4))
        stg = ex.enter_context(tc.tile_pool(name="stg", bufs=3))
        psT = ex.enter_context(tc.tile_pool(name="psT", bufs=4, space="PSUM"))
        psA = ex.enter_context(tc.tile_pool(name="psA", bufs=3, space="PSUM"))
        upl = ex.enter_context(tc.tile_pool(name="upl", bufs=2))
        rpl = ex.enter_context(tc.tile_pool(name="rpl", bufs=2))

        ident = cst.tile([128, 128], f32)
        nc.sync.dma_start(out=ident, in_=id_d.ap())
        wt = cst.tile([F, O], f32)
        nc.sync.dma_start(out=wt, in_=wt_d.ap())

        x_flat = x_d.ap().flatten_outer_dims()  # [b_core*I, F]

        def squash(outp, o_nd, scr, pre_scale):
            """outp [128,25] <- squash of (pre_scale * outp) along d.
            sqrt-free: 1/sqrt(q) = exp(-0.5*ln(q)); Exp/Log/Copy share one
            ACT table set so no table reloads happen anywhere in the kernel.
            """
            sq, sn, r1, qq, r2 = scr
            nc.vector.scalar_tensor_tensor(
                out=sq, in0=outp, scalar=float(pre_scale * pre_scale),
                in1=outp, op0=Alu.mult, op1=Alu.mult)
            nc.vector.tensor_reduce(
                out=sn, in_=sq.rearrange("p (n d) -> p n d", n=N),
                axis=mybir.AxisListType.X, op=Alu.add)
            # r1 = 1/(1+sn)
            nc.vector.tensor_scalar_add(out=r2, in0=sn, scalar1=1.0)
            nc.vector.reciprocal(out=r1, in_=r2)
            # r2 = 1/sqrt(sn + eps) = exp(-0.5 * ln(sn + eps))
            nc.vector.tensor_scalar_add(out=r2, in0=sn, scalar1=float(EPS))
            nc.scalar.activation(qq, r2, Act.Ln)
            nc.scalar.activation(r2, qq, Act.Exp, scale=-0.5)
            # r1 = sn * r1 * r2  (= sn/(1+sn)/sqrt(sn+eps))
            nc.vector.tensor_mul(out=r1, in0=r1, in1=sn)
            nc.vector.tensor_mul(out=r1, in0=r1, in1=r2)
            # outp = (outp * pre_scale) * r1[n] (broadcast over d)
            nc.vector.scalar_tensor_tensor(
                out=o_nd, in0=o_nd, scalar=float(pre_scale),
                in1=r1.broadcast_to((128, N, D)),
                op0=Alu.mult, op1=Alu.mult)

        for blk in range(nblk):
            # ---------------- Phase A: u_hat for 128 examples ----------------
            psa_t = None
            for ch in range(tiles_blk // CH):
                xc = xin.tile([128, CH * F], f32, name="xc")
                base = blk * 128 * I + ch * CH * 128
                # partition p holds rows 2p,2p+1 of each 256-row example:
                # 640B contiguous DRAM runs. Tile a=(e,t): row = e*256+2p+t,
                # so tile a covers example base//256+e with i = 2p+t, which
                # matches the scratch k = 2b+t layout exactly.
                nc.sync.dma_start(
                    out=xc,
                    in_=x_flat[base:base + CH * 128, :]
                    .rearrange("(e p t) f -> p e t f", e=CH // 2, p=128, t=2))
                # all transposes of the chunk first, then all W-matmuls, so
                # the PE never stalls on an eviction mid-chunk
                xts = []
                for g in range(CH // 4):
                    pt = psT.tile([F, 512], f32, name="pt")
                    for q in range(4):
                        t = g * 4 + q
                        nc.tensor.matmul(
                            out=pt[:, q * 128:(q + 1) * 128],
                            lhsT=xc[:, t * F:(t + 1) * F],
                            rhs=ident, start=True, stop=True)
                    xt = xtp.tile([F, 512], f32, name="xt")
                    nc.scalar.copy(out=xt, in_=pt)
                    xts.append(xt)
                for t in range(CH):
                    a = ch * CH + t  # tile index in block
                    slot = a % EV_A
                    if slot == 0:
                        psa_t = psA.tile([128, EV_A * O], f32, name="pa")
                    if a % STG == 0:
                        st = stg.tile([128, STG * O], f32, name="st")
                    nc.tensor.matmul(
                        out=psa_t[:, slot * O:(slot + 1) * O],
                        lhsT=xts[t // 4][:, (t % 4) * 128:(t % 4 + 1) * 128],
                        rhs=wt, start=True, stop=True)
                    if slot == EV_A - 1:
                        g0 = (a % STG) - (EV_A - 1)
                        nc.scalar.copy(
                            out=st[:, g0 * O:(g0 + EV_A) * O], in_=psa_t)
                    if a % STG == STG - 1:
                        k0 = blk * tiles_blk + a - (STG - 1)
                        nc.sync.dma_start(
                            out=us_d.ap()[:, k0:k0 + STG, :], in_=st)

            # ---------------- Phase B: routing for 128 examples --------------
            if stop_after == "phaseA":
                outp0 = rpl.tile([128, O], f32, name="outp0")
                nc.vector.memset(outp0, 0.0)
                nc.sync.dma_start(
                    out=out_d.ap()[blk * 128:(blk + 1) * 128, :], in_=outp0)
                continue
            u_t = upl.tile([128, O * I], f32, name="u_t")
            # lane bb holds example blk*128+bb; per-lane layout (p, h, o):
            # offset (2p+h)*25 + o = i'*25 + o with i' = 2p+h (permuted i,
            # harmless since routing only ever sums over i).
            nc.scalar.dma_start(
                out=u_t.rearrange("b (p h o) -> b p h o", p=128, h=2, o=O),
                in_=us_d.ap()[:, blk * tiles_blk:(blk + 1) * tiles_blk, :]
                .rearrange("p (b h) o -> b p h o", h=2))
            u3 = u_t.rearrange("b (i o) -> b o i", o=O)

            blog = rpl.tile([128, N * I], f32, name="blog")
            e_t = rpl.tile([128, N * I], f32, name="e_t")
            c_t = rpl.tile([128, N * I], f32, name="c_t")
            z_t = rpl.tile([128, I], f32, name="z_t")
            zi_t = rpl.tile([128, I], f32, name="zi_t")
            prod = rpl.tile([128, I], f32, name="prod")
            outp = rpl.tile([128, O], f32, name="outp")
            sq = rpl.tile([128, O], f32, name="sq")
            sn = rpl.tile([128, N], f32, name="sn")
            r1 = rpl.tile([128, N], f32, name="r1")
            qq = rpl.tile([128, N], f32, name="qq")
            r2 = rpl.tile([128, N], f32, name="r2")
            scr = (sq, sn, r1, qq, r2)
            o_nd = outp.rearrange("p (n d) -> p n d", n=N)

            # iter 0: c uniform = 1/5
            nc.vector.tensor_reduce(
                out=outp, in_=u3, axis=mybir.AxisListType.X, op=Alu.add)
            squash(outp, o_nd, scr, 0.2)

            if stop_after == "iter0":
                nc.sync.dma_start(
                    out=out_d.ap()[blk * 128:(blk + 1) * 128, :], in_=outp)
                continue

            for it in range(1, ROUTINGS):
                # blog update with previous outputs
                for o_i in range(O):
                    nn = o_i // D
                    bslice = blog[:, nn * I:(nn + 1) * I]
                    if it == 1 and o_i % D == 0:
                        nc.vector.tensor_scalar(
                            out=bslice, in0=u3[:, o_i, :],
                            scalar1=outp[:, o_i:o_i + 1], scalar2=None,
                            op0=Alu.mult)
                    else:
                        nc.vector.scalar_tensor_tensor(
                            out=bslice, in0=u3[:, o_i, :],
                            scalar=outp[:, o_i:o_i + 1], in1=bslice,
                            op0=Alu.mult, op1=Alu.add)
                # softmax over n (no max subtraction; |blog| is small)
                nc.scalar.activation(e_t, blog, Act.Exp)
                nc.vector.tensor_reduce(
                    out=z_t,
                    in_=e_t.rearrange("b (n i) -> b i n", n=N),
                    axis=mybir.AxisListType.X, op=Alu.add)
                nc.vector.reciprocal(out=zi_t, in_=z_t)
                nc.vector.tensor_tensor(
                    out=c_t.rearrange("b (n i) -> b n i", n=N),
                    in0=e_t.rearrange("b (n i) -> b n i", n=N),
                    in1=zi_t.rearrange("b (one i) -> b one i", one=1)
                    .broadcast_to((128, N, I)),
                    op=Alu.mult)
                # outputs[o] = sum_i c[n, i] * u[o, i]
                for o_i in range(O):
                    nn = o_i // D
                    nc.vector.scalar_tensor_tensor(
                        out=prod, in0=u3[:, o_i, :], scalar=0.0,
                        in1=c_t[:, nn * I:(nn + 1) * I],
                        op0=Alu.bypass, op1=Alu.mult,
                        accum_out=outp[:, o_i:o_i + 1])
                squash(outp, o_nd, scr, 1.0)

            nc.sync.dma_start(
                out=out_d.ap()[blk * 128:(blk + 1) * 128, :], in_=outp)

    nc.finalize()
    return nc


def _get_nc(b_core):
    if b_core not in _CACHE:
        _CACHE[b_core] = _build(b_core)
    return _CACHE[b_core]


LAST_RESULTS = None


def kernel(x, W):
    global LAST_RESULTS
    from concourse import bass_utils
    import os

    x = np.ascontiguousarray(np.asarray(x, dtype=np.float32))
    W = np.asarray(W, dtype=np.float32)
    b_full = x.shape[0]
    b_core = b_full // NCORES
    nc = _get_nc(b_core)

    wt = np.ascontiguousarray(W.T)          # [80, 25]
    ident = np.eye(128, dtype=np.float32)
    in_maps = [
        {"x": x[c * b_core:(c + 1) * b_core], "wt": wt, "ident": ident}
        for c in range(NCORES)
    ]
    trace = bool(int(os.environ.get("KERNEL_TRACE", "0")))
    res = bass_utils.run_bass_kernel_spmd(
        nc, in_maps, list(range(NCORES)), trace=trace)
    LAST_RESULTS = res
    out = np.concatenate([res.results[c]["out"] for c in range(NCORES)], axis=0)
    return out.reshape(b_full, N, D)


if __name__ == "__main__":
    # smoke test against numpy on a small shard
    rng = np.random.default_rng(0)
    x = rng.standard_normal((B_FULL, I, F), dtype=np.float32)
    W = (rng.standard_normal((O, F)) * 0.1).astype(np.float32)
    out = kernel(x, W)
    print("kernel out", out.shape, out.dtype)

